# revision 27
# baseline (speedup 1.0000x reference)
"""KIVI attention wrapper — Trainium2 Bass kernel, 8-way head-sharded. v2.

Sharding: 16 heads / 8 cores = 2 heads per core (tensor parallel) through
attention; c_proj is token-sharded (each core computes the full 1024 output
features for its 512-token slab) fed by per-batch AllToAlls of the per-head
attention outputs.

v2 structural changes vs v1:
  - x is pre-transposed on the host (xT [E, TOK]); stage 1 is a pure GEMM
    (no PE transposes of X, no PSUM->SBUF copies of xT).
  - q/k head-dim feature order is permuted host-side (member j of KIVI
    group g at row 64h+16j+g) so the 2-bit fake-quant of K runs entirely
    in the transposed [feat, tok] layout: group-absmax via two partition-
    shifted abs_max ops, scale broadcast back via one tf32 PE matmul with
    a constant replication matrix. Zero transposes on the K path.
  - the whole pipeline is per-batch: GEMM(b) -> quant(b) -> attention(b)
    -> AllToAll(b), so the first collective fires ~30us in and the
    exchange chain overlaps the remaining batches' compute.
  - per-head EXP (separate PSUM banks per head) to fit all stage pools in
    8 PSUM banks while stages of adjacent batches overlap.
"""
import sys
sys.path.insert(0, '/opt/trn_rl_repo')
import numpy as np

P = 128
TOK = 4096          # B*S = 4*1024
E = 1024
NB = 8              # embed 128-blocks
CH = 512            # token chunk (GEMM granularity)
BT = 1024           # tokens per batch
NBATCH = 4
MAGIC = 8388608.0   # 2^23: x + MAGIC - MAGIC == rint(x) for |x| < 2^22

_CACHE = {}


def _build(sim_single=False):
    import concourse.bacc as bacc
    import concourse.mybir as mybir
    import concourse.tile as tile

    f32 = mybir.dt.float32
    fmm = mybir.dt.float32r
    bf16 = mybir.dt.bfloat16
    ADD = mybir.AluOpType.add
    MULT = mybir.AluOpType.mult
    SUB = mybir.AluOpType.subtract
    MAXOP = mybir.AluOpType.max
    EXP = mybir.ActivationFunctionType.Exp

    nc = bacc.Bacc("TRN2", target_bir_lowering=False, debug=False,
                   num_devices=(1 if sim_single else 8))

    xT_ap = nc.dram_tensor("xT", [E, TOK], bf16, kind="ExternalInput").ap()
    wqkv_ap = nc.dram_tensor("wqkv", [E, 384], bf16, kind="ExternalInput").ap()
    bqkv_ap = nc.dram_tensor("bqkv", [P, 3], f32, kind="ExternalInput").ap()
    m8t_ap = nc.dram_tensor("m8t", [P, 32], f32, kind="ExternalInput").ap()
    wp_ap = nc.dram_tensor("wp", [E, E], bf16, kind="ExternalInput").ap()
    bp_ap = nc.dram_tensor("bp", [P, NB], f32, kind="ExternalInput").ap()
    ident_ap = nc.dram_tensor("ident", [P, P], bf16, kind="ExternalInput").ap()
    ones1_ap = nc.dram_tensor("ones1", [1, 64], fmm, kind="ExternalInput").ap()
    rmat_ap = nc.dram_tensor("rmat", [80, P], fmm, kind="ExternalInput").ap()
    yt_ap = nc.dram_tensor("yt", [NB, P, CH], f32, kind="ExternalOutput").ap()

    with tile.TileContext(nc) as tc:
        with tc.tile_pool(name="const", bufs=1) as constp, \
             tc.tile_pool(name="xp", bufs=1) as xp, \
             tc.tile_pool(name="sb", bufs=2) as sb, \
             tc.tile_pool(name="qnt", bufs=1) as qnt, \
             tc.tile_pool(name="qt", bufs=1) as qtp, \
             tc.tile_pool(name="dram", bufs=1, space="DRAM") as dramp, \
             tc.tile_pool(name="g1ps", bufs=2, space="PSUM") as g1ps, \
             tc.tile_pool(name="s4ps", bufs=2, space="PSUM") as s4ps, \
             tc.tile_pool(name="avps", bufs=2, space="PSUM") as avps, \
             tc.tile_pool(name="rvps", bufs=2, space="PSUM") as rvps:

            # ------------------- constants / weights -------------------
            identb = constp.tile([P, P], bf16)
            nc.sync.dma_start(identb[:], ident_ap)
            wts = []
            for eb in range(NB):
                wt = constp.tile([P, 384], bf16, name=f"wt{eb}", tag=f"wt{eb}")
                nc.sync.dma_start(wt[:], wqkv_ap[eb * P:(eb + 1) * P, :])
                wts.append(wt)
            m8tt = constp.tile([P, 32], f32)
            nc.sync.dma_start(m8tt[:], m8t_ap)
            bqkvt = constp.tile([P, 3], f32)
            nc.sync.dma_start(bqkvt[:], bqkv_ap)
            bpt = constp.tile([P, NB], f32)
            nc.sync.dma_start(bpt[:], bp_ap)
            ones1r = constp.tile([1, 64], fmm)
            nc.sync.dma_start(ones1r[:], ones1_ap)
            rmatt = constp.tile([80, P], fmm)
            nc.sync.dma_start(rmatt[:], rmat_ap)
            onescol = constp.tile([P, 1], bf16)
            nc.any.memset(onescol[:], 1.0)
            # x^T tiles: one per embed block, DMAed per 512-token chunk so
            # the chunk-0 GEMM starts as soon as its 8 slices land
            xns = []
            for eb in range(NB):
                xn = xp.tile([P, TOK], bf16, name=f"xn{eb}", tag=f"xn{eb}")
                xns.append(xn)
            for lo, hi in ((0, CH), (CH, 2 * CH), (2 * CH, 4 * CH),
                           (4 * CH, TOK)):
                for eb in range(NB):
                    nc.sync.dma_start(xns[eb][:, lo:hi],
                                      xT_ap[eb * P:(eb + 1) * P, lo:hi])
            # c_proj weights declared now, loaded mid-pipeline (below)
            wps = []
            for fb in range(NB):
                wpt = constp.tile([P, E], bf16, name=f"wp{fb}", tag=f"wp{fb}")
                wps.append(wpt)

            a2a_ins = [dramp.tile([NB, P, P], bf16, name=f"a2a_in{b}",
                                  tag=f"a2a_in{b}") for b in range(NBATCH)]
            a2a_outs = [dramp.tile([NB, P, P], bf16, name=f"a2a_out{b}",
                                   tag=f"a2a_out{b}") for b in range(NBATCH)]
            # receive buffers: batches 0-2 packed [P, src*384] in one tile
            # so 3/4 of c_proj depends only on the first three exchanges
            recvAall = qtp.tile([P, NB * 3 * P], bf16, name="rAall",
                                tag="rAall")
            recvBall = qtp.tile([P, NB * P], bf16, name="rBall", tag="rBall")

            # ------------------- per-batch pipeline --------------------
            # Software-pipelined emission: engine queues execute in FIFO
            # order, so attention(b-1) is emitted AFTER GEMM(b) — the PE
            # runs batch b's GEMM while batch b-1's quant chain (ACT/DVE/
            # DMA) completes, instead of convoy-stalling on it.
            st_q = [None] * NBATCH   # qT per batch
            st_k = [None] * NBATCH   # kdT per batch
            st_v = [None] * NBATCH   # vts per batch

            def emit_gemm(b):
                qT = sb.tile([P, BT], bf16, name="qT", tag="qT")
                kT = sb.tile([P, BT], bf16, name="kT", tag="kT")
                vT = sb.tile([P, BT], bf16, name="vT", tag="vT")
                qkvT = [qT, kT, vT]
                for qc in range(2):
                    ch = 2 * b + qc
                    cs = slice(ch * CH, (ch + 1) * CH)
                    for m in range(3):
                        gps = g1ps.tile([P, CH], f32, tag="gps")
                        for eb in range(NB):
                            nc.tensor.matmul(
                                gps[:], wts[eb][:, m * P:(m + 1) * P],
                                xns[eb][:, cs],
                                start=(eb == 0), stop=(eb == NB - 1))
                        with nc.allow_low_precision(reason="bf16 store"):
                            nc.vector.tensor_tensor(
                                qkvT[m][:, qc * CH:(qc + 1) * CH], gps[:],
                                bqkvt[:, m:m + 1].to_broadcast((P, CH)),
                                ADD)
                return qT, kT, vT

            def emit_quant_pre(b, kT):
                # KIVI fake-quant of K, feature-major.  Rows are permuted
                # so group-g member j (=2*j1+j0) of head h sits at
                # partition 64h+32*j1+16*j0+g.  DVE ops require operands
                # on identical partitions, so the shifted halves hop
                # through DMA before each abs_max level; two R matmuls
                # (K=16, row groups 0/64) replicate scale=absmax/1.5 back
                # to all 128 rows (tf32).
                aT = qnt.tile([P, BT], bf16, name="aT", tag="aT")
                nc.scalar.activation(aT[:], kT[:],
                                     mybir.ActivationFunctionType.Abs)
                dt = qnt.tile([96, BT], bf16, name="dt", tag="dt")
                nc.sync.dma_start(dt[0:32, :], aT[32:64, :])
                nc.sync.dma_start(dt[64:96, :], aT[96:128, :])
                u = qnt.tile([96, BT], f32, name="u", tag="u")
                nc.vector.tensor_tensor(u[0:32, :], aT[0:32, :],
                                        dt[0:32, :], MAXOP)
                nc.vector.tensor_tensor(u[64:96, :], aT[64:96, :],
                                        dt[64:96, :], MAXOP)
                et = qnt.tile([80, BT], f32, name="et", tag="et")
                nc.sync.dma_start(et[0:16, :], u[16:32, :])
                nc.sync.dma_start(et[64:80, :], u[80:96, :])
                v = qnt.tile([80, BT], fmm, name="v", tag="v")
                with nc.allow_low_precision(reason="tf32 scale basis"):
                    nc.vector.tensor_tensor(v[0:16, :], u[0:16, :],
                                            et[0:16, :], MAXOP)
                    nc.vector.tensor_tensor(v[64:80, :], u[64:80, :],
                                            et[64:80, :], MAXOP)
                return v

            def emit_quant_post(b, kT, v):
                scale = qnt.tile([P, BT], f32, name="scale", tag="scale")
                for j in range(2):
                    js = slice(j * CH, (j + 1) * CH)
                    ps_sc = g1ps.tile([P, CH], f32, tag="gps")
                    nc.tensor.matmul(ps_sc[:], rmatt[0:16, :], v[0:16, js],
                                     start=True, stop=False)
                    nc.tensor.matmul(ps_sc[:], rmatt[64:80, :],
                                     v[64:80, js], start=False, stop=True)
                    nc.scalar.copy(scale[:, js], ps_sc[:])
                rs = qnt.tile([P, BT], f32, name="rs", tag="rs")
                nc.vector.reciprocal_approx_fast(rs[:], scale[:])
                kd = qnt.tile([P, BT], f32, name="kd", tag="kd")
                nc.vector.tensor_tensor(kd[:], kT[:], rs[:], MULT)
                nc.vector.tensor_scalar(kd[:], kd[:], 1.5, MAGIC, ADD, ADD)
                nc.vector.tensor_scalar(kd[:], kd[:], MAGIC, 1.5, SUB, SUB)
                kdT = sb.tile([P, BT], bf16, name="kdT", tag="kdT")
                with nc.allow_low_precision(reason="bf16 store"):
                    nc.vector.tensor_tensor(kdT[:], kd[:], scale[:], MULT)
                return kdT

            def emit_vt(b, vT):
                vts = []
                for j in range(NB):
                    ps_v = rvps.tile([P, P], bf16, tag="rv")
                    nc.tensor.transpose(ps_v[:], vT[:, j * P:(j + 1) * P],
                                        identb[:])
                    vh = []
                    for h in range(2):
                        vt = sb.tile([P, 65], bf16, name=f"v{j}_{h}",
                                     tag=f"v{j}_{h}")
                        if (j + h) % 2 == 0:
                            nc.scalar.copy(vt[:, 0:64],
                                           ps_v[:, h * 64:(h + 1) * 64])
                        else:
                            nc.vector.tensor_copy(
                                vt[:, 0:64], ps_v[:, h * 64:(h + 1) * 64])
                        nc.vector.tensor_copy(vt[:, 64:65], onescol[:])
                        vh.append(vt)
                    vts.append(vh)
                return vts

            def emit_attn_a2a(b):
                qT, kdT, vts = st_q[b], st_k[b], st_v[b]
                oT = sb.tile([P, BT], bf16, name="oT", tag="oT")
                for qc in range(2):
                    qs = slice(qc * CH, (qc + 1) * CH)
                    es = []
                    for kb in range(NB):
                        e = sb.tile([P, 2 * CH], bf16, name=f"e{kb}",
                                    tag=f"e{kb}")
                        for h in range(2):
                            hs = slice(h * 64, (h + 1) * 64)
                            ps_s = s4ps.tile([P, CH], f32, tag="ps_s")
                            nc.tensor.matmul(
                                ps_s[:], kdT[hs, kb * P:(kb + 1) * P],
                                qT[hs, qs], start=True, stop=True)
                            nc.scalar.activation(
                                e[:, h * CH:(h + 1) * CH], ps_s[:], EXP,
                                bias=m8tt[:, b * 8 + kb:b * 8 + kb + 1],
                                scale=0.125)
                        es.append(e)
                    for h in range(2):
                        hs = slice(h * 64, (h + 1) * 64)
                        ps_av = avps.tile([65, CH], f32, tag="ps_av")
                        for kb in range(NB):
                            nc.tensor.matmul(
                                ps_av[:], vts[kb][h][:],
                                es[kb][:, h * CH:(h + 1) * CH],
                                start=(kb == 0), stop=(kb == NB - 1))
                        denS = sb.tile([1, CH], fmm, name="denS",
                                       tag="denS")
                        with nc.allow_low_precision(reason="tf32 copy"):
                            nc.vector.tensor_copy(denS[:], ps_av[64:65, :])
                        ps_r = rvps.tile([64, CH], f32, tag="rv")
                        nc.tensor.matmul(ps_r[:], ones1r[:], denS[:],
                                         start=True, stop=True)
                        rrep = sb.tile([64, CH], f32, name="rrep",
                                       tag="rrep")
                        nc.vector.reciprocal_approx_fast(rrep[:], ps_r[:])
                        with nc.allow_low_precision(reason="bf16 store"):
                            nc.vector.tensor_tensor(
                                oT[hs, qs], ps_av[0:64, :], rrep[:], MULT)
                for j in range(NB):
                    nc.sync.dma_start(a2a_ins[b][j],
                                      oT[:, j * P:(j + 1) * P])
                if sim_single:
                    for r in range(NB):
                        nc.gpsimd.dma_start(a2a_outs[b][r], a2a_ins[b][r])
                else:
                    nc.gpsimd.collective_compute(
                        "AllToAll", mybir.AluOpType.bypass,
                        replica_groups=[list(range(8))],
                        ins=[a2a_ins[b][:]], outs=[a2a_outs[b][:]])
                for s in range(NB):
                    if b < 3:
                        nc.sync.dma_start(
                            recvAall[:, s * 3 * P + b * P:
                                     s * 3 * P + (b + 1) * P],
                            a2a_outs[b][s])
                    else:
                        nc.sync.dma_start(recvBall[:, s * P:(s + 1) * P],
                                          a2a_outs[b][s])

            for b in range(NBATCH):
                qT, kT, vT = emit_gemm(b)
                if b == 1:
                    for fb in range(NB):
                        nc.sync.dma_start(wps[fb][:],
                                          wp_ap[fb * P:(fb + 1) * P, :])
                st_q[b] = qT
                v = emit_quant_pre(b, kT)
                st_v[b] = emit_vt(b, vT)
                if b >= 1:
                    emit_attn_a2a(b - 1)
                # R-matmuls sit after attention(b-1) in the PE FIFO so the
                # quant-chain latency never stalls the PE
                st_k[b] = emit_quant_post(b, kT, v)
            emit_attn_a2a(NBATCH - 1)

            # ------------- stage 5: token-sharded c_proj ---------------
            # PSUM slots reuse the GEMM pool's tag (stage-1 is done by now)
            with tc.tile_pool(name="s5", bufs=2) as s5p:
                W = 3 * P
                for eb in range(NB):
                    ps_p = g1ps.tile([P, W], f32, tag="gps")
                    for fb in range(NB):
                        nc.tensor.matmul(ps_p[:],
                                         wps[fb][:, eb * P:(eb + 1) * P],
                                         recvAall[:, fb * W:(fb + 1) * W],
                                         start=(fb == 0), stop=(fb == NB - 1))
                    yts = s5p.tile([P, W], f32, name=f"ytsA{eb}", tag="ytsA")
                    nc.vector.tensor_tensor(
                        yts[:], ps_p[:],
                        bpt[:, eb:eb + 1].to_broadcast((P, W)), ADD)
                    nc.sync.dma_start(yt_ap[eb][:, 0:W], yts[:])
                for eb in range(NB):
                    ps_p = g1ps.tile([P, CH - W], f32, tag="gps")
                    for fb in range(NB):
                        nc.tensor.matmul(ps_p[:],
                                         wps[fb][:, eb * P:(eb + 1) * P],
                                         recvBall[:, fb * P:(fb + 1) * P],
                                         start=(fb == 0), stop=(fb == NB - 1))
                    yts = s5p.tile([P, CH - W], f32, name=f"ytsB{eb}",
                                   tag="ytsB")
                    nc.vector.tensor_tensor(
                        yts[:], ps_p[:],
                        bpt[:, eb:eb + 1].to_broadcast((P, CH - W)), ADD)
                    nc.sync.dma_start(yt_ap[eb][:, W:CH], yts[:])

    nc.compile()
    return nc


def _perm_idx():
    """Permutation: new row 64h+32*j1+16*j0+g holds feature 64h+4g+j,
    j = 2*j1+j0."""
    idx = np.empty(128, dtype=np.int64)
    for h in range(2):
        for j in range(4):
            j1, j0 = j >> 1, j & 1
            for g in range(16):
                idx[64 * h + 32 * j1 + 16 * j0 + g] = 64 * h + 4 * g + j
    return idx


def make_in_maps(hidden_states, attention_mask, w_attn, b_attn, w_proj, b_proj):
    import ml_dtypes
    bf = ml_dtypes.bfloat16
    x = np.asarray(hidden_states, np.float32).reshape(TOK, E)
    xT = np.ascontiguousarray(x.T).astype(bf)
    mask = np.asarray(attention_mask, np.float32)
    wa = np.asarray(w_attn, np.float32)
    ba = np.asarray(b_attn, np.float32)
    wpf = np.ascontiguousarray(np.asarray(w_proj, np.float32)).astype(bf)
    bp = np.asarray(b_proj, np.float32)

    m8 = (mask * np.float32(0.125)).reshape(4, 8, 128)
    m8t = np.ascontiguousarray(m8.transpose(2, 0, 1).reshape(128, 32))
    ident = np.eye(P, dtype=bf)
    ones1 = np.ones((1, 64), dtype=np.float32)
    bp_pack = np.ascontiguousarray(bp.reshape(NB, P).T)
    idx = _perm_idx()
    rmat = np.zeros((80, P), dtype=np.float32)
    for h in range(2):
        for g in range(16):
            for j in range(4):
                j1, j0 = j >> 1, j & 1
                rmat[64 * h + g,
                     64 * h + 32 * j1 + 16 * j0 + g] = 1.0 / 1.5

    in_maps = []
    for c in range(8):
        cs = slice(c * P, (c + 1) * P)
        wq = wa[:, cs][:, idx]
        wk = wa[:, 1024 + c * P:1024 + (c + 1) * P][:, idx]
        wv = wa[:, 2048 + c * P:2048 + (c + 1) * P]
        wqkv = np.ascontiguousarray(
            np.concatenate([wq, wk, wv], axis=1)).astype(bf)
        bqkv = np.ascontiguousarray(np.stack(
            [ba[cs][idx], ba[1024 + c * P:1024 + (c + 1) * P][idx],
             ba[2048 + c * P:2048 + (c + 1) * P]], axis=1))
        in_maps.append({
            "xT": xT, "wqkv": wqkv, "bqkv": bqkv, "m8t": m8t,
            "wp": wpf, "bp": bp_pack, "ident": ident, "ones1": ones1,
            "rmat": rmat,
        })
    return in_maps


def kernel(hidden_states, attention_mask, w_attn, b_attn, w_proj, b_proj):
    from concourse import bass_utils
    if "nc" not in _CACHE:
        _CACHE["nc"] = _build()
    nc = _CACHE["nc"]
    in_maps = make_in_maps(hidden_states, attention_mask, w_attn, b_attn,
                           w_proj, b_proj)
    res = bass_utils.run_bass_kernel_spmd(nc, in_maps, core_ids=list(range(8)))
    y = np.empty((TOK, E), dtype=np.float32)
    for c in range(8):
        blk = res.results[c]["yt"].reshape(E, 4, P)  # [feat, batch, tok]
        for b in range(4):
            y[b * 1024 + c * P:b * 1024 + (c + 1) * P, :] = blk[:, b, :].T
    return y.reshape(4, 1024, E)


# revision 28
# speedup vs baseline: 1.0263x; 1.0263x over previous
"""KIVI attention wrapper — Trainium2 Bass kernel, 8-way head-sharded. v2.

Sharding: 16 heads / 8 cores = 2 heads per core (tensor parallel) through
attention; c_proj is token-sharded (each core computes the full 1024 output
features for its 512-token slab) fed by per-batch AllToAlls of the per-head
attention outputs.

v2 structural changes vs v1:
  - x is pre-transposed on the host (xT [E, TOK]); stage 1 is a pure GEMM
    (no PE transposes of X, no PSUM->SBUF copies of xT).
  - q/k head-dim feature order is permuted host-side (member j of KIVI
    group g at row 64h+16j+g) so the 2-bit fake-quant of K runs entirely
    in the transposed [feat, tok] layout: group-absmax via two partition-
    shifted abs_max ops, scale broadcast back via one tf32 PE matmul with
    a constant replication matrix. Zero transposes on the K path.
  - the whole pipeline is per-batch: GEMM(b) -> quant(b) -> attention(b)
    -> AllToAll(b), so the first collective fires ~30us in and the
    exchange chain overlaps the remaining batches' compute.
  - per-head EXP (separate PSUM banks per head) to fit all stage pools in
    8 PSUM banks while stages of adjacent batches overlap.
"""
import sys
sys.path.insert(0, '/opt/trn_rl_repo')
import numpy as np

P = 128
TOK = 4096          # B*S = 4*1024
E = 1024
NB = 8              # embed 128-blocks
CH = 512            # token chunk (GEMM granularity)
BT = 1024           # tokens per batch
NBATCH = 4
MAGIC = 8388608.0   # 2^23: x + MAGIC - MAGIC == rint(x) for |x| < 2^22

_CACHE = {}


def _build(sim_single=False):
    import concourse.bacc as bacc
    import concourse.mybir as mybir
    import concourse.tile as tile

    f32 = mybir.dt.float32
    fmm = mybir.dt.float32r
    bf16 = mybir.dt.bfloat16
    ADD = mybir.AluOpType.add
    MULT = mybir.AluOpType.mult
    SUB = mybir.AluOpType.subtract
    MAXOP = mybir.AluOpType.max
    EXP = mybir.ActivationFunctionType.Exp

    nc = bacc.Bacc("TRN2", target_bir_lowering=False, debug=False,
                   num_devices=(1 if sim_single else 8))

    xT_ap = nc.dram_tensor("xT", [E, TOK], bf16, kind="ExternalInput").ap()
    wqkv_ap = nc.dram_tensor("wqkv", [E, 384], bf16, kind="ExternalInput").ap()
    bqkv_ap = nc.dram_tensor("bqkv", [P, 3], f32, kind="ExternalInput").ap()
    m8t_ap = nc.dram_tensor("m8t", [P, 32], f32, kind="ExternalInput").ap()
    wp_ap = nc.dram_tensor("wp", [E, E], bf16, kind="ExternalInput").ap()
    bp_ap = nc.dram_tensor("bp", [P, NB], f32, kind="ExternalInput").ap()
    ident_ap = nc.dram_tensor("ident", [P, P], bf16, kind="ExternalInput").ap()
    ones1_ap = nc.dram_tensor("ones1", [1, 64], fmm, kind="ExternalInput").ap()
    rmat_ap = nc.dram_tensor("rmat", [80, P], fmm, kind="ExternalInput").ap()
    yt_ap = nc.dram_tensor("yt", [NB, P, CH], f32, kind="ExternalOutput").ap()

    with tile.TileContext(nc) as tc:
        with tc.tile_pool(name="const", bufs=1) as constp, \
             tc.tile_pool(name="xp", bufs=1) as xp, \
             tc.tile_pool(name="sb", bufs=2) as sb, \
             tc.tile_pool(name="qnt", bufs=1) as qnt, \
             tc.tile_pool(name="qt", bufs=1) as qtp, \
             tc.tile_pool(name="dram", bufs=1, space="DRAM") as dramp, \
             tc.tile_pool(name="g1ps", bufs=2, space="PSUM") as g1ps, \
             tc.tile_pool(name="s4ps", bufs=2, space="PSUM") as s4ps, \
             tc.tile_pool(name="avps", bufs=2, space="PSUM") as avps, \
             tc.tile_pool(name="rvps", bufs=2, space="PSUM") as rvps:

            # ------------------- constants / weights -------------------
            identb = constp.tile([P, P], bf16)
            nc.sync.dma_start(identb[:], ident_ap)
            wts = []
            for eb in range(NB):
                wt = constp.tile([P, 384], bf16, name=f"wt{eb}", tag=f"wt{eb}")
                nc.sync.dma_start(wt[:], wqkv_ap[eb * P:(eb + 1) * P, :])
                wts.append(wt)
            m8tt = constp.tile([P, 32], f32)
            nc.sync.dma_start(m8tt[:], m8t_ap)
            bqkvt = constp.tile([P, 3], f32)
            nc.sync.dma_start(bqkvt[:], bqkv_ap)
            bpt = constp.tile([P, NB], f32)
            nc.sync.dma_start(bpt[:], bp_ap)
            ones1r = constp.tile([1, 64], fmm)
            nc.sync.dma_start(ones1r[:], ones1_ap)
            rmatt = constp.tile([80, P], fmm)
            nc.sync.dma_start(rmatt[:], rmat_ap)
            onescol = constp.tile([P, 1], bf16)
            nc.any.memset(onescol[:], 1.0)
            # x^T tiles: one per embed block, DMAed per 512-token chunk so
            # the chunk-0 GEMM starts as soon as its 8 slices land
            xns = []
            for eb in range(NB):
                xn = xp.tile([P, TOK], bf16, name=f"xn{eb}", tag=f"xn{eb}")
                xns.append(xn)
            for lo, hi in ((0, CH), (CH, 2 * CH), (2 * CH, 4 * CH)):
                for eb in range(NB):
                    nc.sync.dma_start(xns[eb][:, lo:hi],
                                      xT_ap[eb * P:(eb + 1) * P, lo:hi])
            # c_proj weights declared now, loaded mid-pipeline (below)
            wps = []
            for fb in range(NB):
                wpt = constp.tile([P, E], bf16, name=f"wp{fb}", tag=f"wp{fb}")
                wps.append(wpt)

            a2a_ins = [dramp.tile([NB, P, P], bf16, name=f"a2a_in{b}",
                                  tag=f"a2a_in{b}") for b in range(NBATCH)]
            a2a_outs = [dramp.tile([NB, P, P], bf16, name=f"a2a_out{b}",
                                   tag=f"a2a_out{b}") for b in range(NBATCH)]
            # receive buffers: batches 0-2 packed [P, src*384] in one tile
            # so 3/4 of c_proj depends only on the first three exchanges
            recvAall = qtp.tile([P, NB * 3 * P], bf16, name="rAall",
                                tag="rAall")
            recvBall = qtp.tile([P, NB * P], bf16, name="rBall", tag="rBall")

            # ------------------- per-batch pipeline --------------------
            # Software-pipelined emission: engine queues execute in FIFO
            # order, so attention(b-1) is emitted AFTER GEMM(b) — the PE
            # runs batch b's GEMM while batch b-1's quant chain (ACT/DVE/
            # DMA) completes, instead of convoy-stalling on it.
            st_q = [None] * NBATCH   # qT per batch
            st_k = [None] * NBATCH   # kdT per batch
            st_v = [None] * NBATCH   # vts per batch

            def emit_gemm(b):
                qT = sb.tile([P, BT], bf16, name="qT", tag="qT")
                kT = sb.tile([P, BT], bf16, name="kT", tag="kT")
                vT = sb.tile([P, BT], bf16, name="vT", tag="vT")
                qkvT = [qT, kT, vT]
                for qc in range(2):
                    ch = 2 * b + qc
                    cs = slice(ch * CH, (ch + 1) * CH)
                    for m in range(3):
                        gps = g1ps.tile([P, CH], f32, tag="gps")
                        for eb in range(NB):
                            nc.tensor.matmul(
                                gps[:], wts[eb][:, m * P:(m + 1) * P],
                                xns[eb][:, cs],
                                start=(eb == 0), stop=(eb == NB - 1))
                        with nc.allow_low_precision(reason="bf16 store"):
                            nc.vector.tensor_tensor(
                                qkvT[m][:, qc * CH:(qc + 1) * CH], gps[:],
                                bqkvt[:, m:m + 1].to_broadcast((P, CH)),
                                ADD)
                return qT, kT, vT

            def emit_quant_pre(b, kT):
                # KIVI fake-quant of K, feature-major.  Rows are permuted
                # so group-g member j (=2*j1+j0) of head h sits at
                # partition 64h+32*j1+16*j0+g.  DVE ops require operands
                # on identical partitions, so the shifted halves hop
                # through DMA before each abs_max level; two R matmuls
                # (K=16, row groups 0/64) replicate scale=absmax/1.5 back
                # to all 128 rows (tf32).
                aT = qnt.tile([P, BT], bf16, name="aT", tag="aT")
                nc.scalar.activation(aT[:], kT[:],
                                     mybir.ActivationFunctionType.Abs)
                dt = qnt.tile([96, BT], bf16, name="dt", tag="dt")
                nc.sync.dma_start(dt[0:32, :], aT[32:64, :])
                nc.sync.dma_start(dt[64:96, :], aT[96:128, :])
                u = qnt.tile([96, BT], f32, name="u", tag="u")
                nc.vector.tensor_tensor(u[0:32, :], aT[0:32, :],
                                        dt[0:32, :], MAXOP)
                nc.vector.tensor_tensor(u[64:96, :], aT[64:96, :],
                                        dt[64:96, :], MAXOP)
                et = qnt.tile([80, BT], f32, name="et", tag="et")
                nc.sync.dma_start(et[0:16, :], u[16:32, :])
                nc.sync.dma_start(et[64:80, :], u[80:96, :])
                v = qnt.tile([80, BT], fmm, name="v", tag="v")
                with nc.allow_low_precision(reason="tf32 scale basis"):
                    nc.vector.tensor_tensor(v[0:16, :], u[0:16, :],
                                            et[0:16, :], MAXOP)
                    nc.vector.tensor_tensor(v[64:80, :], u[64:80, :],
                                            et[64:80, :], MAXOP)
                return v

            def emit_quant_post(b, kT, v):
                scale = qnt.tile([P, BT], f32, name="scale", tag="scale")
                for j in range(2):
                    js = slice(j * CH, (j + 1) * CH)
                    ps_sc = g1ps.tile([P, CH], f32, tag="gps")
                    nc.tensor.matmul(ps_sc[:], rmatt[0:16, :], v[0:16, js],
                                     start=True, stop=False)
                    nc.tensor.matmul(ps_sc[:], rmatt[64:80, :],
                                     v[64:80, js], start=False, stop=True)
                    nc.scalar.copy(scale[:, js], ps_sc[:])
                rs = qnt.tile([P, BT], f32, name="rs", tag="rs")
                nc.vector.reciprocal_approx_fast(rs[:], scale[:])
                kd = qnt.tile([P, BT], f32, name="kd", tag="kd")
                nc.vector.tensor_tensor(kd[:], kT[:], rs[:], MULT)
                nc.vector.tensor_scalar(kd[:], kd[:], 1.5, MAGIC, ADD, ADD)
                nc.vector.tensor_scalar(kd[:], kd[:], MAGIC, 1.5, SUB, SUB)
                kdT = sb.tile([P, BT], bf16, name="kdT", tag="kdT")
                with nc.allow_low_precision(reason="bf16 store"):
                    nc.vector.tensor_tensor(kdT[:], kd[:], scale[:], MULT)
                return kdT

            def emit_vt(b, vT):
                vts = []
                for j in range(NB):
                    ps_v = rvps.tile([P, P], bf16, tag="rv")
                    nc.tensor.transpose(ps_v[:], vT[:, j * P:(j + 1) * P],
                                        identb[:])
                    vh = []
                    for h in range(2):
                        vt = sb.tile([P, 65], bf16, name=f"v{j}_{h}",
                                     tag=f"v{j}_{h}")
                        if (j + h) % 2 == 0:
                            nc.scalar.copy(vt[:, 0:64],
                                           ps_v[:, h * 64:(h + 1) * 64])
                        else:
                            nc.vector.tensor_copy(
                                vt[:, 0:64], ps_v[:, h * 64:(h + 1) * 64])
                        nc.vector.tensor_copy(vt[:, 64:65], onescol[:])
                        vh.append(vt)
                    vts.append(vh)
                return vts

            def emit_attn_a2a(b):
                qT, kdT, vts = st_q[b], st_k[b], st_v[b]
                oT = sb.tile([P, BT], bf16, name="oT", tag="oT")
                for qc in range(2):
                    qs = slice(qc * CH, (qc + 1) * CH)
                    es = []
                    for kb in range(NB):
                        e = sb.tile([P, 2 * CH], bf16, name=f"e{kb}",
                                    tag=f"e{kb}")
                        for h in range(2):
                            hs = slice(h * 64, (h + 1) * 64)
                            ps_s = s4ps.tile([P, CH], f32, tag="ps_s")
                            nc.tensor.matmul(
                                ps_s[:], kdT[hs, kb * P:(kb + 1) * P],
                                qT[hs, qs], start=True, stop=True)
                            nc.scalar.activation(
                                e[:, h * CH:(h + 1) * CH], ps_s[:], EXP,
                                bias=m8tt[:, b * 8 + kb:b * 8 + kb + 1],
                                scale=0.125)
                        es.append(e)
                    for h in range(2):
                        hs = slice(h * 64, (h + 1) * 64)
                        ps_av = avps.tile([65, CH], f32, tag="ps_av")
                        for kb in range(NB):
                            nc.tensor.matmul(
                                ps_av[:], vts[kb][h][:],
                                es[kb][:, h * CH:(h + 1) * CH],
                                start=(kb == 0), stop=(kb == NB - 1))
                        denS = sb.tile([1, CH], fmm, name="denS",
                                       tag="denS")
                        with nc.allow_low_precision(reason="tf32 copy"):
                            nc.vector.tensor_copy(denS[:], ps_av[64:65, :])
                        ps_r = rvps.tile([64, CH], f32, tag="rv")
                        nc.tensor.matmul(ps_r[:], ones1r[:], denS[:],
                                         start=True, stop=True)
                        rrep = sb.tile([64, CH], f32, name="rrep",
                                       tag="rrep")
                        nc.vector.reciprocal_approx_fast(rrep[:], ps_r[:])
                        with nc.allow_low_precision(reason="bf16 store"):
                            nc.vector.tensor_tensor(
                                oT[hs, qs], ps_av[0:64, :], rrep[:], MULT)
                for j in range(NB):
                    nc.sync.dma_start(a2a_ins[b][j],
                                      oT[:, j * P:(j + 1) * P])
                if sim_single:
                    for r in range(NB):
                        nc.gpsimd.dma_start(a2a_outs[b][r], a2a_ins[b][r])
                else:
                    nc.gpsimd.collective_compute(
                        "AllToAll", mybir.AluOpType.bypass,
                        replica_groups=[list(range(8))],
                        ins=[a2a_ins[b][:]], outs=[a2a_outs[b][:]])
                for s in range(NB):
                    if b < 3:
                        nc.sync.dma_start(
                            recvAall[:, s * 3 * P + b * P:
                                     s * 3 * P + (b + 1) * P],
                            a2a_outs[b][s])
                    else:
                        nc.sync.dma_start(recvBall[:, s * P:(s + 1) * P],
                                          a2a_outs[b][s])

            for b in range(NBATCH):
                if b == 1:
                    # bulk tail of x (chunks 4-7, needed from batch 2 on)
                    # queues behind batch-0's latency-critical hop DMAs
                    for eb in range(NB):
                        nc.sync.dma_start(
                            xns[eb][:, 4 * CH:TOK],
                            xT_ap[eb * P:(eb + 1) * P, 4 * CH:TOK])
                qT, kT, vT = emit_gemm(b)
                if b == 1:
                    for fb in range(NB):
                        nc.sync.dma_start(wps[fb][:],
                                          wp_ap[fb * P:(fb + 1) * P, :])
                st_q[b] = qT
                v = emit_quant_pre(b, kT)
                st_v[b] = emit_vt(b, vT)
                if b >= 1:
                    emit_attn_a2a(b - 1)
                # R-matmuls sit after attention(b-1) in the PE FIFO so the
                # quant-chain latency never stalls the PE
                st_k[b] = emit_quant_post(b, kT, v)
            emit_attn_a2a(NBATCH - 1)

            # ------------- stage 5: token-sharded c_proj ---------------
            # PSUM slots reuse the GEMM pool's tag (stage-1 is done by now)
            with tc.tile_pool(name="s5", bufs=2) as s5p:
                W = 3 * P
                for eb in range(NB):
                    ps_p = g1ps.tile([P, W], f32, tag="gps")
                    for fb in range(NB):
                        nc.tensor.matmul(ps_p[:],
                                         wps[fb][:, eb * P:(eb + 1) * P],
                                         recvAall[:, fb * W:(fb + 1) * W],
                                         start=(fb == 0), stop=(fb == NB - 1))
                    yts = s5p.tile([P, W], f32, name=f"ytsA{eb}", tag="ytsA")
                    nc.vector.tensor_tensor(
                        yts[:], ps_p[:],
                        bpt[:, eb:eb + 1].to_broadcast((P, W)), ADD)
                    nc.sync.dma_start(yt_ap[eb][:, 0:W], yts[:])
                for eb in range(NB):
                    ps_p = g1ps.tile([P, CH - W], f32, tag="gps")
                    for fb in range(NB):
                        nc.tensor.matmul(ps_p[:],
                                         wps[fb][:, eb * P:(eb + 1) * P],
                                         recvBall[:, fb * P:(fb + 1) * P],
                                         start=(fb == 0), stop=(fb == NB - 1))
                    yts = s5p.tile([P, CH - W], f32, name=f"ytsB{eb}",
                                   tag="ytsB")
                    nc.vector.tensor_tensor(
                        yts[:], ps_p[:],
                        bpt[:, eb:eb + 1].to_broadcast((P, CH - W)), ADD)
                    nc.sync.dma_start(yt_ap[eb][:, W:CH], yts[:])

    nc.compile()
    return nc


def _perm_idx():
    """Permutation: new row 64h+32*j1+16*j0+g holds feature 64h+4g+j,
    j = 2*j1+j0."""
    idx = np.empty(128, dtype=np.int64)
    for h in range(2):
        for j in range(4):
            j1, j0 = j >> 1, j & 1
            for g in range(16):
                idx[64 * h + 32 * j1 + 16 * j0 + g] = 64 * h + 4 * g + j
    return idx


def make_in_maps(hidden_states, attention_mask, w_attn, b_attn, w_proj, b_proj):
    import ml_dtypes
    bf = ml_dtypes.bfloat16
    x = np.asarray(hidden_states, np.float32).reshape(TOK, E)
    xT = np.ascontiguousarray(x.T).astype(bf)
    mask = np.asarray(attention_mask, np.float32)
    wa = np.asarray(w_attn, np.float32)
    ba = np.asarray(b_attn, np.float32)
    wpf = np.ascontiguousarray(np.asarray(w_proj, np.float32)).astype(bf)
    bp = np.asarray(b_proj, np.float32)

    m8 = (mask * np.float32(0.125)).reshape(4, 8, 128)
    m8t = np.ascontiguousarray(m8.transpose(2, 0, 1).reshape(128, 32))
    ident = np.eye(P, dtype=bf)
    ones1 = np.ones((1, 64), dtype=np.float32)
    bp_pack = np.ascontiguousarray(bp.reshape(NB, P).T)
    idx = _perm_idx()
    rmat = np.zeros((80, P), dtype=np.float32)
    for h in range(2):
        for g in range(16):
            for j in range(4):
                j1, j0 = j >> 1, j & 1
                rmat[64 * h + g,
                     64 * h + 32 * j1 + 16 * j0 + g] = 1.0 / 1.5

    in_maps = []
    for c in range(8):
        cs = slice(c * P, (c + 1) * P)
        wq = wa[:, cs][:, idx]
        wk = wa[:, 1024 + c * P:1024 + (c + 1) * P][:, idx]
        wv = wa[:, 2048 + c * P:2048 + (c + 1) * P]
        wqkv = np.ascontiguousarray(
            np.concatenate([wq, wk, wv], axis=1)).astype(bf)
        bqkv = np.ascontiguousarray(np.stack(
            [ba[cs][idx], ba[1024 + c * P:1024 + (c + 1) * P][idx],
             ba[2048 + c * P:2048 + (c + 1) * P]], axis=1))
        in_maps.append({
            "xT": xT, "wqkv": wqkv, "bqkv": bqkv, "m8t": m8t,
            "wp": wpf, "bp": bp_pack, "ident": ident, "ones1": ones1,
            "rmat": rmat,
        })
    return in_maps


def kernel(hidden_states, attention_mask, w_attn, b_attn, w_proj, b_proj):
    from concourse import bass_utils
    if "nc" not in _CACHE:
        _CACHE["nc"] = _build()
    nc = _CACHE["nc"]
    in_maps = make_in_maps(hidden_states, attention_mask, w_attn, b_attn,
                           w_proj, b_proj)
    res = bass_utils.run_bass_kernel_spmd(nc, in_maps, core_ids=list(range(8)))
    y = np.empty((TOK, E), dtype=np.float32)
    for c in range(8):
        blk = res.results[c]["yt"].reshape(E, 4, P)  # [feat, batch, tok]
        for b in range(4):
            y[b * 1024 + c * P:b * 1024 + (c + 1) * P, :] = blk[:, b, :].T
    return y.reshape(4, 1024, E)
